# revision 1
# baseline (speedup 1.0000x reference)
"""Trainium2 Bass kernel for nn_DecoderForGeoLossLess (3-level sparse
transposed-conv LSTM decoder, 5000 -> 20000 -> 80000 -> 320000 voxels).

Strategy (v3): parent chains composed on host; each level-2 node s gets
(x0-row, k0, x1-row, k1) so phase A computes hx1/cx1 per s-slot with no
inter-level gather.  Phase A runs as three loops (t0 / t1 / transpose)
so PSUM tags can double-buffer; DVE tails are software-pipelined one
pair behind the activations so ACT (the throughput bound) never idles.
The hx1/cx1 table lives in SBUF in dma_gather-native layout (slot c ->
partition c%128, byte offset (c//128)*256).  Phase B dma-gathers parent
rows (4096 idxs per gather), runs the fused LSTM cell, writes bf16 hx2.
All matmuls bf16 operands, f32 PSUM.  No collectives.
"""

import os
import numpy as np
import ml_dtypes

import concourse.bass as bass
import concourse.mybir as mybir
import concourse.tile as tile
from concourse import bacc
from concourse.masks import make_identity
from concourse.bass_utils import run_bass_kernel_spmd

F32 = mybir.dt.float32
BF16 = mybir.dt.bfloat16
I16 = mybir.dt.int16
BF = ml_dtypes.bfloat16
SIG = mybir.ActivationFunctionType.Sigmoid
TANH = mybir.ActivationFunctionType.Tanh

N0, N1, N2, N3 = 5000, 20000, 80000, 320000
HID = 64
K = 8
NCORES = 8
CH = 512            # chunk columns (one PSUM bank of f32)
PAIR = 2 * CH
GATH = int(os.environ.get("KGRP", "4096"))  # idxs per gather group


def _round_up(x, m):
    return (x + m - 1) // m * m


def _assign_balanced(cell_s, childcnt):
    """Assign each s-node to a core, keeping per-(k1,k0)-cell counts equal
    (+-0) across cores and balancing total child count (j load)."""
    core_of_s = np.empty(N2, np.int8)
    j_load = np.zeros(NCORES, np.int64)
    cell_pad = np.zeros(64, np.int64)
    for cell in range(64):
        ss = np.nonzero(cell_s == cell)[0]
        order = ss[np.argsort(-childcnt[ss], kind="stable")]
        cell_pad[cell] = (len(ss) + NCORES - 1) // NCORES
        pos = 0
        while pos < len(order):
            grp = order[pos:pos + NCORES]
            cores = np.argsort(j_load, kind="stable")[:len(grp)]
            core_of_s[grp] = cores
            j_load[cores] += childcnt[grp]
            pos += NCORES
    return core_of_s, cell_pad


def _segments(bounds_k, lo, hi):
    """Static segment list [(st, en, k)] intersecting [lo, hi) with the
    monotone boundary table bounds_k = list of (end_pos, k, start_pos)."""
    segs = []
    for end_pos, kk, start_pos in bounds_k:
        st = max(lo, start_pos)
        en = min(hi, end_pos)
        if st < en:
            segs.append((st - lo, en - lo, kk))
    return segs


def _prepare(x0, x1, x2, W_i, W_h, W_c,
             parent0, kofs0, parent1, kofs1, parent2, kofs2):
    s_par = parent2.astype(np.int64)
    k2j = kofs2.astype(np.int64)
    r_of_s = parent1.astype(np.int64)
    k1s = kofs1.astype(np.int64)
    q_of_s = parent0.astype(np.int64)[r_of_s]
    k0s = kofs0.astype(np.int64)[r_of_s]

    childcnt = np.bincount(s_par, minlength=N2)
    cell_s = k1s * K + k0s
    core_of_s, cell_pad = _assign_balanced(cell_s, childcnt)

    NS_real = int(cell_pad.sum())
    NS_pad = _round_up(NS_real, PAIR)
    cell_pad[63] += NS_pad - NS_real
    cell_off = np.concatenate([[0], np.cumsum(cell_pad)])

    # per-core s slots
    spos = np.zeros(N2, np.int64)
    s_slot_lists = []   # per core: (slots, s_ids)
    for c in range(NCORES):
        slots_all, s_all = [], []
        for cell in range(64):
            ss = np.nonzero((cell_s == cell) & (core_of_s == c))[0]
            slots = np.arange(cell_off[cell], cell_off[cell] + len(ss))
            spos[ss] = slots
            slots_all.append(slots)
            s_all.append(ss)
        s_slot_lists.append((np.concatenate(slots_all), np.concatenate(s_all)))

    # j buckets by k2, shared padded sizes
    j_core = core_of_s[s_par]
    cnt = np.zeros((NCORES, K), np.int64)
    for c in range(NCORES):
        cnt[c] = np.bincount(k2j[j_core == c], minlength=K)
    b2 = cnt.max(axis=0)
    NJ_real = int(b2.sum())
    NJ_pad = _round_up(NJ_real, GATH)
    b2[K - 1] += NJ_pad - NJ_real
    boff = np.concatenate([[0], np.cumsum(b2)])

    j_slot_lists = []   # per core: (slots, j_ids)
    for c in range(NCORES):
        slots_all, j_all = [], []
        for b in range(K):
            jj = np.nonzero((j_core == c) & (k2j == b))[0]
            slots_all.append(np.arange(boff[b], boff[b] + len(jj)))
            j_all.append(jj)
        j_slot_lists.append((np.concatenate(slots_all), np.concatenate(j_all)))

    # device input arrays
    in_maps = []
    for c in range(NCORES):
        sslots, sids = s_slot_lists[c]
        jslots, jids = j_slot_lists[c]
        X0T = np.zeros((64, NS_pad), BF)
        X0T[:, sslots] = x0[q_of_s[sids]].astype(BF).T
        X1T = np.zeros((64, NS_pad), BF)
        X1T[:, sslots] = x1[r_of_s[sids]].astype(BF).T
        X2T = np.zeros((64, NJ_pad), BF)
        X2T[:, jslots] = x2[s_par[jids]].astype(BF).T
        gflat = np.zeros(NJ_pad, np.int16)
        gflat[jslots] = spos[s_par[jids]].astype(np.int16)
        gidx = np.tile(gflat.reshape(NJ_pad // 16, 16).T, (8, 1))
        in_maps.append({"X0T": X0T, "X1T": X1T, "X2T": X2T,
                        "GIDX": np.ascontiguousarray(gidx)})

    # packed weights (shared across cores); gate order [i, c, f, o]
    # original split order: in 0:64, f 64:128, c 128:192, o 192:256
    perm = np.concatenate([np.arange(0, 64), np.arange(128, 192),
                           np.arange(64, 128), np.arange(192, 256)])
    W0 = np.zeros((64, K * 192), BF)
    W1 = np.zeros((128, K * 256), BF)
    W1s = np.zeros((128, K * 256), BF)
    WC = np.zeros((128, K * 64), BF)
    for k in range(K):
        wi = W_i[k].astype(BF)
        wh = W_h[k].astype(BF)
        W0[:, 192 * k:192 * k + 64] = wi[:, 0:64]
        W0[:, 192 * k + 64:192 * k + 128] = wi[:, 128:192]
        W0[:, 192 * k + 128:192 * k + 192] = wi[:, 192:256]
        W1[0:64, 256 * k:256 * (k + 1)] = wh[:, perm]
        W1[64:128, 256 * k:256 * (k + 1)] = wi[:, perm]
        W1s[0:64, 256 * k:256 * (k + 1)] = wi[:, perm]
        W1s[64:128, 256 * k:256 * (k + 1)] = wh[:, perm]
        WC[0:64, 64 * k:64 * (k + 1)] = W_c[k].astype(BF)
        WC[64:128, 64 * k:64 * (k + 1)] = W_c[k].astype(BF)
    for m in in_maps:
        m.update({"W0": W0, "W1": W1, "W1s": W1s, "WC": WC})

    # baked segment tables
    cellb = [(int(cell_off[c + 1]), (c % K, c // K), int(cell_off[c]))
             for c in range(64)]            # (end, (k0, k1), start)
    t0b = [(e, kk[0], s) for (e, kk, s) in cellb]
    # k1 runs: cells grouped by k1 (cells are k1-major)
    t1b = []
    for k1 in range(K):
        st = int(cell_off[k1 * K])
        en = int(cell_off[(k1 + 1) * K])
        t1b.append((en, k1, st))
    t2b = [(int(boff[b + 1]), b, int(boff[b])) for b in range(K)]

    meta = dict(NS_pad=NS_pad, NJ_pad=NJ_pad,
                t0b=t0b, t1b=t1b, t2b=t2b, j_slot_lists=j_slot_lists)
    return in_maps, meta


def _build(meta):
    NS_pad, NJ_pad = meta["NS_pad"], meta["NJ_pad"]
    t0b, t1b, t2b = meta["t0b"], meta["t1b"], meta["t2b"]

    nc = bacc.Bacc("TRN2", target_bir_lowering=False, debug=False,
                   num_devices=NCORES,
                   dynamic_dma_scratch_size=int(
                       os.environ.get("KSCR", "16384")))
    X0T = nc.dram_tensor("X0T", [64, NS_pad], BF16, kind="ExternalInput")
    X1T = nc.dram_tensor("X1T", [64, NS_pad], BF16, kind="ExternalInput")
    X2T = nc.dram_tensor("X2T", [64, NJ_pad], BF16, kind="ExternalInput")
    GIDX = nc.dram_tensor("GIDX", [128, NJ_pad // 16], I16, kind="ExternalInput")
    W0 = nc.dram_tensor("W0", [64, K * 192], BF16, kind="ExternalInput")
    W1 = nc.dram_tensor("W1", [128, K * 256], BF16, kind="ExternalInput")
    W1s = nc.dram_tensor("W1s", [128, K * 256], BF16, kind="ExternalInput")
    WC = nc.dram_tensor("WC", [128, K * 64], BF16, kind="ExternalInput")
    OUT = nc.dram_tensor("OUT", [128, NJ_pad // 2], BF16, kind="ExternalOutput")

    _ph = os.environ.get("KPARTS", "012B")
    nA = NS_pad // PAIR
    nB = NJ_pad // PAIR if "B" in _ph else 0
    _do0, _do1, _do2 = "0" in _ph, "1" in _ph, "2" in _ph
    _x0whole = os.environ.get("KX0", "sliced") == "whole"

    with tile.TileContext(nc) as tc:
        with (
            tc.tile_pool(name="const", bufs=1) as cst,
            tc.tile_pool(name="dram", bufs=1, space="DRAM") as dpool,
            tc.tile_pool(name="sa", bufs=2) as sa,
            tc.tile_pool(name="sk", bufs=nA) as sk,
            tc.tile_pool(name="sr", bufs=14) as sr,
            tc.tile_pool(name="sb", bufs=2) as sbp,
        ):
            # w0 + x0sb first: phase A0 needs only these two.  x0 comes in
            # per-pair slices so the first t0 matmuls start early.
            w0 = cst.tile([64, K * 192], BF16)
            nc.sync.dma_start(w0[:], W0[:])
            x0sb = cst.tile([64, NS_pad], BF16)
            if _x0whole:
                nc.sync.dma_start(x0sb[:], X0T[:])
            else:
                for p in range(nA):
                    nc.sync.dma_start(x0sb[:, p * PAIR:(p + 1) * PAIR],
                                      X0T[:, p * PAIR:(p + 1) * PAIR])
            w1 = cst.tile([128, K * 256], BF16)
            nc.sync.dma_start(w1[:], W1[:])
            w1s = cst.tile([128, K * 256], BF16)
            nc.sync.dma_start(w1s[:], W1s[:])
            wc = cst.tile([128, K * 64], BF16)
            nc.sync.dma_start(wc[:], WC[:])
            gidx = cst.tile([128, NJ_pad // 16], I16)
            nc.sync.dma_start(gidx[:], GIDX[:])
            idt = cst.tile([128, 64], BF16)
            make_identity(nc, idt[0:64, :])
            make_identity(nc, idt[64:128, :])

            # gather-native table: slot c -> partition c%128, bytes
            # [(c//128)*256, +256) = elems [(c//128)*128, +128)
            tbl = cst.tile([128, NS_pad], BF16)
            _hbm_gather = os.environ.get("KGATHER", "sbuf") == "hbm"
            TBLD = dpool.tile([NS_pad, 128], BF16, name="TBLD") if _hbm_gather else None

            # cross-loop per-pair state (phase A)
            r1as = [sk.tile([128, CH], BF16, tag="r1a", name=f"r1a{p}")
                    for p in range(nA)]
            r1bs = [sk.tile([128, CH], BF16, tag="r1b", name=f"r1b{p}")
                    for p in range(nA)]
            cx0s = [sk.tile([128, CH], BF16, tag="cx0", name=f"cx0{p}")
                    for p in range(nA)]
            hx1s = [sk.tile([128, CH], BF16, tag="hx1", name=f"hx1{p}")
                    for p in range(nA)]
            cxts = [sk.tile([128, CH], BF16, tag="cxt", name=f"cxt{p}")
                    for p in range(nA)]
            for p in range(nA):
                w_lo = p * PAIR
                nc.sync.dma_start(r1as[p][64:128, :], X1T[:, w_lo:w_lo + CH])
                nc.sync.dma_start(r1bs[p][0:64, :],
                                  X1T[:, w_lo + CH:w_lo + PAIR])

            # ---------------- A0: t0 ------------------------------------
            s0s = [None] * nA

            def _a0_tail(p):
                s0 = s0s[p]
                nc.vector.tensor_mul(cx0s[p][:], s0[:, 0:CH], s0[:, CH:PAIR])
                t0t = sa.tile([128, CH], BF16, tag="t0t")
                nc.scalar.activation(t0t[:], cx0s[p][:], TANH)
                nc.vector.tensor_mul(r1as[p][0:64, :], s0[0:64, PAIR:PAIR + CH],
                                     t0t[0:64, :])
                nc.vector.tensor_mul(r1bs[p][64:128, :],
                                     s0[64:128, PAIR:PAIR + CH], t0t[64:128, :])

            with tc.tile_pool(name="p0", bufs=2, space="PSUM") as p0:
                for p in range(nA if _do0 else 0):
                    w_lo = p * PAIR
                    t0g = p0.tile([128, PAIR + CH], F32, tag="t0")
                    for side in range(2):
                        pb = 64 * side
                        segs = _segments(t0b, w_lo + side * CH,
                                         w_lo + (side + 1) * CH)
                        for st, en, k0 in segs:
                            for g in range(3):
                                nc.tensor.matmul(
                                    t0g[pb:pb + 64, g * CH + st:g * CH + en],
                                    w0[:, 192 * k0 + 64 * g:192 * k0 + 64 * (g + 1)],
                                    x0sb[:, w_lo + side * CH + st:
                                         w_lo + side * CH + en],
                                    start=True, stop=True, tile_position=(0, pb))
                    s0 = sa.tile([128, PAIR + CH], BF16, tag="s0")
                    # two acts: a PSUM-read AP must not span >2 banks on HW
                    nc.scalar.activation(s0[:, 0:PAIR], t0g[:, 0:PAIR], SIG)
                    nc.scalar.activation(s0[:, PAIR:PAIR + CH],
                                         t0g[:, PAIR:PAIR + CH], SIG)
                    s0s[p] = s0
                    if p > 0:
                        _a0_tail(p - 1)
                if _do0:
                    _a0_tail(nA - 1)

            # gather groups: runs of <=4 pairs, never straddling the table
            # half boundary (each half has its own src slice + local idxs)
            gpp = GATH // PAIR
            groups = []                     # (start_pair, n_pairs)
            q = 0
            while q < nB:
                n = min(gpp, nB - q)
                groups.append((q, n))
                q += n
            nG = len(groups)
            pair2grp = np.zeros(max(nB, 1), np.int64)
            for gi, (q, n) in enumerate(groups):
                pair2grp[q:q + n] = gi
            hxgs = {}

            def _gather(gi):
                """Gather group gi's parent rows from the table."""
                q, n = groups[gi]
                num = n * PAIR
                hxg = sbp.tile([128, 1, num], BF16, tag="hxg", bufs=3,
                               name=f"hxg{gi}", padded_shape=[128, 1, GATH])
                gsz = int(os.environ.get("KGSZ", "512"))
                for c0 in range(0, num, gsz):
                    cn = min(gsz, num - c0)
                    dst = hxg[:, :, c0:c0 + cn]
                    isl = gidx[:, (q * PAIR + c0) // 16:
                               (q * PAIR + c0 + cn) // 16]
                    if _hbm_gather:
                        nc.gpsimd.dma_gather(
                            dst, TBLD[:, :], isl, cn, cn, 128, transpose=True)
                    else:
                        nc.gpsimd.dma_gather(
                            dst, tbl[:, :], isl, cn, cn, 128, transpose=True,
                            sbuf_tokens_per_rank=128,
                            sbuf_free_dim_per_rank=256)
                hxgs[gi] = hxg

            # ---------------- A1: t1 + transpose into table -------------
            sabs = [None] * nA
            cxucs = [None] * nA

            def _a1_tail(p):
                sA, sB = sabs[p]
                ppr = sa.tile([128, CH], BF16, tag="ppr")
                nc.vector.tensor_mul(ppr[:], sA[:, 0:CH], sA[:, CH:PAIR])
                qq = sa.tile([128, CH], BF16, tag="qq")
                nc.vector.tensor_mul(qq[:], sB[:, 0:CH], cxucs[p][:])
                nc.vector.tensor_add(cxts[p][:], ppr[:], qq[:])
                t1t = sa.tile([128, CH], BF16, tag="t1t")
                nc.scalar.activation(t1t[:], cxts[p][:], TANH)
                nc.vector.tensor_mul(hx1s[p][:], sB[:, CH:PAIR], t1t[:])

            with (
                tc.tile_pool(name="p1a", bufs=2, space="PSUM") as p1a,
                tc.tile_pool(name="p1z", bufs=2, space="PSUM") as p1z,
            ):
                for p in range(nA if _do1 else 0):
                    w_lo = p * PAIR
                    cxu = p1z.tile([128, CH], F32, tag="z")
                    gA = p1a.tile([128, PAIR], F32, tag="a")
                    gB = p1z.tile([128, PAIR], F32, tag="z")
                    for side in range(2):
                        pb = 64 * side
                        r1 = r1as[p] if side == 0 else r1bs[p]
                        wg = w1 if side == 0 else w1s
                        segs = _segments(t1b, w_lo + side * CH,
                                         w_lo + (side + 1) * CH)
                        for st, en, k1 in segs:
                            nc.tensor.matmul(
                                cxu[pb:pb + 64, st:en],
                                wc[pb:pb + 64, 64 * k1:64 * (k1 + 1)],
                                cx0s[p][pb:pb + 64, st:en], start=True,
                                stop=True, tile_position=(pb, pb))
                            for g in range(2):
                                nc.tensor.matmul(
                                    gA[pb:pb + 64, g * CH + st:g * CH + en],
                                    wg[:, 256 * k1 + 64 * g:256 * k1 + 64 * (g + 1)],
                                    r1[:, st:en], start=True, stop=True,
                                    tile_position=(0, pb))
                                nc.tensor.matmul(
                                    gB[pb:pb + 64, g * CH + st:g * CH + en],
                                    wg[:, 256 * k1 + 128 + 64 * g:
                                       256 * k1 + 128 + 64 * (g + 1)],
                                    r1[:, st:en], start=True, stop=True,
                                    tile_position=(0, pb))
                    sA = sa.tile([128, PAIR], BF16, tag="sA")
                    nc.scalar.activation(sA[:], gA[:], SIG)
                    sB = sa.tile([128, PAIR], BF16, tag="sB")
                    nc.scalar.activation(sB[:], gB[:], SIG)
                    sabs[p] = (sA, sB)
                    cxuc = sa.tile([128, CH], BF16, tag="cxuc")
                    nc.vector.tensor_copy(cxuc[:], cxu[:])
                    cxucs[p] = cxuc
                    if p > 0:
                        _a1_tail(p - 1)
                if _do1:
                    _a1_tail(nA - 1)

            def _x2dma(p):
                r2 = sr.tile([128, PAIR], BF16, tag="r2", name=f"r2_{p}")
                nc.sync.dma_start(r2[64:128, :],
                                  X2T[:, p * PAIR:(p + 1) * PAIR])
                return r2

            X2_AHEAD = 12
            r2s = {p: _x2dma(p) for p in range(min(X2_AHEAD, nB))}

            # ---------------- A2: transpose into table ------------------
            with tc.tile_pool(name="ptp", bufs=4, space="PSUM") as ptp:
                for p in range(nA if _do2 else 0):
                    w_lo = p * PAIR
                    for side in range(2):
                        pb = 64 * side
                        tp = (pb, 0) if side else None
                        for b in range(CH // 128):
                            kk = 4 * side + b
                            pt = ptp.tile([128, 128], BF16, tag="pt")
                            nc.tensor.transpose(
                                pt[:, 0:64],
                                hx1s[p][pb:pb + 64, 128 * b:128 * (b + 1)],
                                idt[pb:pb + 64, :], tile_position=tp)
                            nc.tensor.transpose(
                                pt[:, 64:128],
                                cxts[p][pb:pb + 64, 128 * b:128 * (b + 1)],
                                idt[pb:pb + 64, :], tile_position=tp)
                            # DVE only: GPSIMD cannot read PSUM
                            nc.vector.tensor_copy(
                                tbl[:, w_lo + 128 * kk:w_lo + 128 * (kk + 1)],
                                pt[:])
                    if _hbm_gather:
                        # row-major DRAM mirror of this pair's table region
                        dst = TBLD[w_lo:w_lo + PAIR, :].rearrange(
                            "(bk q) f -> q bk f", q=128)
                        nc.sync.dma_start(
                            dst, tbl[:, w_lo:w_lo + PAIR].rearrange(
                                "p (bk f) -> p bk f", bk=8))
                    if p == nA - 1 and nB:
                        # table complete: start the first gathers
                        for gi in range(min(2, nG)):
                            _gather(gi)

            # ---------------- phase B: t2 ------------------------------
            with (
                tc.tile_pool(name="p2a", bufs=2, space="PSUM") as p2ap,
                tc.tile_pool(name="p2z", bufs=2, space="PSUM") as p2z,
            ):
                state = {}      # p -> (sB, cxuc2, w_lo)

                def _b_tail(p):
                    sA, sB, cxuc2, w_lo = state.pop(p)
                    ppr2 = sbp.tile([128, CH], BF16, tag="ppr2")
                    nc.vector.tensor_mul(ppr2[:], sA[:, 0:CH], sA[:, CH:PAIR])
                    qq2 = sbp.tile([128, CH], BF16, tag="qq2")
                    nc.vector.tensor_mul(qq2[:], sB[:, 0:CH], cxuc2[:])
                    cxf = sbp.tile([128, CH], BF16, tag="cxf")
                    nc.vector.tensor_add(cxf[:], ppr2[:], qq2[:])
                    t2t = sbp.tile([128, CH], BF16, tag="t2t")
                    nc.scalar.activation(t2t[:], cxf[:], TANH)
                    hxo = sbp.tile([128, CH], BF16, tag="hxo")
                    nc.vector.tensor_mul(hxo[:], sB[:, CH:PAIR], t2t[:])
                    # OUT via Pool SWDGE keeps the in-order SP queue free
                    # for the early X2T loads; KOUT=sp falls back to SP
                    _oeng = (nc.sync if os.environ.get("KOUT", "pool") == "sp"
                             else nc.gpsimd)
                    _oeng.dma_start(OUT[:, w_lo // 2:w_lo // 2 + CH], hxo[:])

                for p in range(nB):
                    w_lo = p * PAIR
                    gi = int(pair2grp[p])
                    g_start, g_n = groups[gi]
                    if p == g_start and gi + 2 < nG:
                        _gather(gi + 2)
                    hxg = hxgs[gi]
                    if p == g_start + g_n - 1:
                        hxgs.pop(gi)
                    off = (p - g_start) * PAIR
                    if p + X2_AHEAD < nB:
                        r2s[p + X2_AHEAD] = _x2dma(p + X2_AHEAD)
                    r2 = r2s.pop(p)
                    nc.vector.tensor_copy(r2[0:64, :],
                                          hxg[0:64, 0, off:off + PAIR])

                    segs2 = [_segments(t2b, w_lo + s * CH, w_lo + (s + 1) * CH)
                             for s in range(2)]
                    cxu2 = p2z.tile([128, CH], F32, tag="z")
                    for side in range(2):
                        pb = 64 * side
                        for st, en, k2 in segs2[side]:
                            nc.tensor.matmul(
                                cxu2[pb:pb + 64, st:en],
                                wc[64:128, 64 * k2:64 * (k2 + 1)],
                                hxg[64:128, 0, off + side * CH + st:
                                    off + side * CH + en],
                                start=True, stop=True, tile_position=(64, pb))

                    g2a = p2ap.tile([128, PAIR], F32, tag="a")
                    g2b = p2z.tile([128, PAIR], F32, tag="z")
                    for side in range(2):
                        pb = 64 * side
                        for st, en, k2 in segs2[side]:
                            for g in range(2):
                                nc.tensor.matmul(
                                    g2a[pb:pb + 64, g * CH + st:g * CH + en],
                                    w1[:, 256 * k2 + 64 * g:256 * k2 + 64 * (g + 1)],
                                    r2[:, side * CH + st:side * CH + en],
                                    start=True, stop=True, tile_position=(0, pb))
                                nc.tensor.matmul(
                                    g2b[pb:pb + 64, g * CH + st:g * CH + en],
                                    w1[:, 256 * k2 + 128 + 64 * g:
                                       256 * k2 + 128 + 64 * (g + 1)],
                                    r2[:, side * CH + st:side * CH + en],
                                    start=True, stop=True, tile_position=(0, pb))
                    s2a = sbp.tile([128, PAIR], BF16, tag="s2a")
                    nc.scalar.activation(s2a[:], g2a[:], SIG)
                    s2b = sbp.tile([128, PAIR], BF16, tag="s2b")
                    nc.scalar.activation(s2b[:], g2b[:], SIG)
                    cxuc2 = sbp.tile([128, CH], BF16, tag="cxuc2")
                    nc.vector.tensor_copy(cxuc2[:], cxu2[:])
                    state[p] = (s2a, s2b, cxuc2, w_lo)
                    if p > 0:
                        _b_tail(p - 1)
                if nB:
                    _b_tail(nB - 1)

    nc.finalize()
    return nc


LAST_NC = None


def _run(inputs, trace=False):
    global LAST_NC
    in_maps, meta = _prepare(**inputs)
    nc = _build(meta)
    LAST_NC = nc
    res = run_bass_kernel_spmd(nc, in_maps, core_ids=list(range(NCORES)),
                               trace=trace)
    NJ_pad = meta["NJ_pad"]
    out = np.zeros((N3, 64), np.float32)
    for c in range(NCORES):
        jslots, jids = meta["j_slot_lists"][c]
        oc = res.results[c]["OUT"]          # [128, NJ_pad//2] bf16 packed pairs
        # column q*512+t at rows [h*64:(h+1)*64] holds j-slot q*1024+h*512+t
        flat = oc.reshape(2, 64, NJ_pad // PAIR, CH).transpose(1, 2, 0, 3)
        flat = np.ascontiguousarray(flat).reshape(64, NJ_pad)
        out[jids] = flat[:, jslots].T.astype(np.float32)
    return out, res


def kernel(**inputs):
    out, _ = _run(inputs, trace=False)
    return out



# revision 17
# speedup vs baseline: 1.1826x; 1.1826x over previous
"""Trainium2 Bass kernel for nn_DecoderForGeoLossLess (3-level sparse
transposed-conv LSTM decoder, 5000 -> 20000 -> 80000 -> 320000 voxels).

Strategy (v5, node-major + overlapped phases): parent chains composed on
host; each level-2 slot s gets (x0-row, k0, x1-row, k1).  Phase A0
computes the t0 LSTM feature-major.  Phases A1/B run NODE-MAJOR: a
128-node block of [feat, node] activations is the matmul STATIONARY and
the weights are the moving operand, so every gate of a node lands on one
PSUM partition.  Gates come from two accumulating matmuls (hx-part +
x-part), so no [hx;x] tile assembly is needed anywhere.  A1 tails write
hx/cx straight into the gather-native table.  A1 and B share one
gates[128,1024]x3 + cxu[128,256]x2 PSUM pool pair, so there is no
psum barrier between the phases.  Phase B j-slots are bucketed by
(parent-quartile, k2) and sorted by parent slot, and each dma_gather
chunk's source AP is sliced to the baked parent-slot prefix it needs, so
gathers (4096-idx chunks) overlap the tail of A1.  tanh is batched
across groups and deferred two groups so the DVE queue never
head-of-line blocks on it.  ACT (sigmoid/tanh) is the throughput floor;
all sig instrs are 2-PSUM-bank [128,1024] reads.  All matmuls bf16
operands, f32 PSUM.  No collectives.
"""

import os
import numpy as np
import ml_dtypes

import concourse.bass as bass
import concourse.mybir as mybir
import concourse.tile as tile
from concourse import bacc
from concourse.bass_utils import run_bass_kernel_spmd

F32 = mybir.dt.float32
BF16 = mybir.dt.bfloat16
I16 = mybir.dt.int16
BF = ml_dtypes.bfloat16
SIG = mybir.ActivationFunctionType.Sigmoid
TANH = mybir.ActivationFunctionType.Tanh

N0, N1, N2, N3 = 5000, 20000, 80000, 320000
HID = 64
K = 8
NCORES = 8
CH = 512            # A0 side width / A1+B group width (slots)
PAIR = 2 * CH       # A0 pair width
BLK = 128           # nodes per stationary block
GB = 4              # blocks per group
NQ = 4              # parent-quartile sub-buckets per k2 bucket
GATH = int(os.environ.get("KGRP", "4096"))   # idxs per gather chunk
TBAT = int(os.environ.get("KTB", "8"))       # B tanh batch (groups)
DLY = int(os.environ.get("KDLY", "2"))       # tanh batch deferral (groups)
STAGE_G = 4                                   # groups per OUT stage chunk


def _round_up(x, m):
    return (x + m - 1) // m * m


def _assign_balanced(cell_s, childcnt, k2cnt):
    """Assign each s-node to a core, keeping per-(k1,k0)-cell counts equal
    (+-0) across cores and balancing per-(core,k2) child counts (the max
    over cores per k2 bucket sets the padded phase-B size)."""
    core_of_s = np.empty(N2, np.int8)
    bload = np.zeros((NCORES, K), np.float64)   # per-core per-k2 j load
    cell_pad = np.zeros(64, np.int64)
    for cell in range(64):
        ss = np.nonzero(cell_s == cell)[0]
        order = ss[np.argsort(-childcnt[ss], kind="stable")]
        cell_pad[cell] = (len(ss) + NCORES - 1) // NCORES
        pos = 0
        while pos < len(order):
            grp = order[pos:pos + NCORES]
            avail = list(range(NCORES))
            for s in grp:
                a = k2cnt[s]
                # sum-of-squares cost pushes each k2 bucket toward balance
                costs = [(((bload[c] + a) ** 2).sum(), c) for c in avail]
                costs.sort()
                c = costs[0][1]
                core_of_s[s] = c
                bload[c] += a
                avail.remove(c)
            pos += NCORES
    return core_of_s, cell_pad


def _segments(bounds_k, lo, hi):
    segs = []
    for end_pos, kk, start_pos in bounds_k:
        st = max(lo, start_pos)
        en = min(hi, end_pos)
        if st < en:
            segs.append((st - lo, en - lo, kk))
    return segs


def _prepare(x0, x1, x2, W_i, W_h, W_c,
             parent0, kofs0, parent1, kofs1, parent2, kofs2):
    s_par = parent2.astype(np.int64)
    k2j = kofs2.astype(np.int64)
    r_of_s = parent1.astype(np.int64)
    k1s = kofs1.astype(np.int64)
    q_of_s = parent0.astype(np.int64)[r_of_s]
    k0s = kofs0.astype(np.int64)[r_of_s]

    childcnt = np.bincount(s_par, minlength=N2)
    k2cnt = np.zeros((N2, K), np.int64)
    np.add.at(k2cnt, (s_par, k2j), 1)
    cell_s = k1s * K + k0s
    core_of_s, cell_pad = _assign_balanced(cell_s, childcnt, k2cnt)

    # align every k1-group (8 cells) to BLK so A1 blocks are k1-pure,
    # then the total to PAIR for A0's pair structure
    for k1 in range(K):
        tot = int(cell_pad[k1 * K:(k1 + 1) * K].sum())
        cell_pad[k1 * K + K - 1] += _round_up(tot, BLK) - tot
    NS_real = int(cell_pad.sum())
    NS_pad = _round_up(NS_real, PAIR)
    cell_pad[63] += NS_pad - NS_real
    cell_off = np.concatenate([[0], np.cumsum(cell_pad)])

    spos = np.zeros(N2, np.int64)
    s_slot_lists = []
    for c in range(NCORES):
        slots_all, s_all = [], []
        for cell in range(64):
            ss = np.nonzero((cell_s == cell) & (core_of_s == c))[0]
            slots = np.arange(cell_off[cell], cell_off[cell] + len(ss))
            spos[ss] = slots
            slots_all.append(slots)
            s_all.append(ss)
        s_slot_lists.append((np.concatenate(slots_all), np.concatenate(s_all)))

    # ---- j slots: buckets by k2, BLK-aligned; each bucket split into NQ
    # parent-quartile sub-buckets (cut at BLK-aligned indices, no extra
    # padding); j's sorted by parent slot within each bucket.  Slot order
    # is quartile-major so early gather chunks only touch low table slots.
    j_core = core_of_s[s_par]
    cnt = np.zeros((NCORES, K), np.int64)
    for c in range(NCORES):
        cnt[c] = np.bincount(k2j[j_core == c], minlength=K)
    b2 = np.array([_round_up(int(v), BLK) for v in cnt.max(axis=0)], np.int64)
    NJ_real = int(b2.sum())
    NJ_pad = _round_up(NJ_real, STAGE_G * GB * BLK)
    b2[K - 1] += NJ_pad - NJ_real

    # shared sub-bucket sizes: split b2[b]//BLK blocks into NQ parts
    sb_sz = np.zeros((K, NQ), np.int64)
    for b in range(K):
        m = b2[b] // BLK
        for q in range(NQ):
            sb_sz[b, q] = (m // NQ + (1 if q < m % NQ else 0)) * BLK
    # slot offsets, quartile-major then bucket
    sb_off = np.zeros((K, NQ), np.int64)
    pos = 0
    k2_of_block = np.zeros(NJ_pad // BLK, np.int64)
    for q in range(NQ):
        for b in range(K):
            sb_off[b, q] = pos
            k2_of_block[pos // BLK:(pos + sb_sz[b, q]) // BLK] = b
            pos += sb_sz[b, q]
    assert pos == NJ_pad

    j_slot_lists = []
    gflats = []
    for c in range(NCORES):
        slots_all, j_all = [], []
        for b in range(K):
            jj = np.nonzero((j_core == c) & (k2j == b))[0]
            jj = jj[np.argsort(spos[s_par[jj]], kind="stable")]
            taken = 0
            for q in range(NQ):
                n = min(int(sb_sz[b, q]), len(jj) - taken)
                if n <= 0:
                    continue
                slots_all.append(np.arange(sb_off[b, q], sb_off[b, q] + n))
                j_all.append(jj[taken:taken + n])
                taken += n
        jslots = np.concatenate(slots_all)
        jids = np.concatenate(j_all)
        j_slot_lists.append((jslots, jids))
        gflat = np.zeros(NJ_pad, np.int16)
        gflat[jslots] = spos[s_par[jids]].astype(np.int16)
        gflats.append(gflat)

    # per-gather-chunk parent-slot prefix limit (shared across cores)
    chunk_lims = []
    q = 0
    while q < NJ_pad:
        n = min(GATH, NJ_pad - q)
        mx = max(int(g[q:q + n].max()) for g in gflats)
        chunk_lims.append(_round_up(mx + 1, BLK))
        q += n

    in_maps = []
    for c in range(NCORES):
        sslots, sids = s_slot_lists[c]
        X0T = np.zeros((64, NS_pad), BF)
        X0T[:, sslots] = x0[q_of_s[sids]].astype(BF).T
        X1T = np.zeros((64, NS_pad), BF)
        X1T[:, sslots] = x1[r_of_s[sids]].astype(BF).T
        # parity-packed x1: even 512-groups at rows 0:64, odd at 64:128,
        # cols indexed by pair (mirrors hx1sb) so odd A1 groups' x-part
        # matmul shares the hx-part's tile position
        X1P = np.zeros((128, NS_pad // 2), BF)
        x1v = X1T.reshape(64, NS_pad // 1024, 2, 512)
        X1P[0:64] = x1v[:, :, 0, :].reshape(64, NS_pad // 2)
        X1P[64:128] = x1v[:, :, 1, :].reshape(64, NS_pad // 2)
        jslots, jids = j_slot_lists[c]
        X2T = np.zeros((64, NJ_pad), BF)
        X2T[:, jslots] = x2[s_par[jids]].astype(BF).T
        gidx = np.tile(gflats[c].reshape(NJ_pad // 16, 16).T, (8, 1))
        in_maps.append({"X0T": X0T, "X1P": X1P, "X2T": X2T,
                        "GIDX": np.ascontiguousarray(gidx)})

    # packed weights; natural gate order [i, f, c, o].  A0 only needs
    # [i, c, o] (hx=0, cx=0 at t=0).
    W0 = np.zeros((64, K * 192), BF)
    WHD = np.zeros((128, K * 256), BF)   # W_h duplicated in both halves
    W1I = np.zeros((128, K * 256), BF)   # W_i duplicated in both halves
    WC2 = np.zeros((128, K * 64), BF)    # W_c duplicated in both halves
    for k in range(K):
        wi = W_i[k].astype(BF)
        wh = W_h[k].astype(BF)
        W0[:, 192 * k:192 * k + 64] = wi[:, 0:64]           # i
        W0[:, 192 * k + 64:192 * k + 128] = wi[:, 128:192]  # c
        W0[:, 192 * k + 128:192 * k + 192] = wi[:, 192:256]  # o
        WHD[0:64, 256 * k:256 * (k + 1)] = wh
        WHD[64:128, 256 * k:256 * (k + 1)] = wh
        W1I[0:64, 256 * k:256 * (k + 1)] = wi
        W1I[64:128, 256 * k:256 * (k + 1)] = wi
        WC2[0:64, 64 * k:64 * (k + 1)] = W_c[k].astype(BF)
        WC2[64:128, 64 * k:64 * (k + 1)] = W_c[k].astype(BF)
    for m in in_maps:
        m.update({"W0": W0, "WHD": WHD, "W1I": W1I, "WC2": WC2})

    t0b = [(int(cell_off[c + 1]), c % K, int(cell_off[c]))
           for c in range(64)]
    k1_of_block = np.zeros(NS_pad // BLK, np.int64)
    for k1 in range(K):
        st, en = int(cell_off[k1 * K]), int(cell_off[(k1 + 1) * K])
        k1_of_block[st // BLK:en // BLK] = k1

    meta = dict(NS_pad=NS_pad, NJ_pad=NJ_pad, t0b=t0b,
                k1_of_block=[int(v) for v in k1_of_block],
                k2_of_block=[int(v) for v in k2_of_block],
                chunk_lims=chunk_lims,
                j_slot_lists=j_slot_lists)
    return in_maps, meta


def _r3(ap, b):
    """[128, b*64] AP -> [128, b, 64] view."""
    return ap.rearrange("p (b f) -> p b f", b=b)


def _build(meta):
    NS_pad, NJ_pad = meta["NS_pad"], meta["NJ_pad"]
    t0b = meta["t0b"]
    k1b = meta["k1_of_block"]
    k2b = meta["k2_of_block"]
    lims = meta["chunk_lims"]

    nc = bacc.Bacc("TRN2", target_bir_lowering=False, debug=False,
                   num_devices=NCORES,
                   dynamic_dma_scratch_size=int(
                       os.environ.get("KSCR", "16384")))
    X0T = nc.dram_tensor("X0T", [64, NS_pad], BF16, kind="ExternalInput")
    X1P = nc.dram_tensor("X1P", [128, NS_pad // 2], BF16,
                        kind="ExternalInput")
    X2T = nc.dram_tensor("X2T", [64, NJ_pad], BF16, kind="ExternalInput")
    GIDX = nc.dram_tensor("GIDX", [128, NJ_pad // 16], I16, kind="ExternalInput")
    W0 = nc.dram_tensor("W0", [64, K * 192], BF16, kind="ExternalInput")
    WHD = nc.dram_tensor("WHD", [128, K * 256], BF16, kind="ExternalInput")
    W1I = nc.dram_tensor("W1I", [128, K * 256], BF16, kind="ExternalInput")
    WC2 = nc.dram_tensor("WC2", [128, K * 64], BF16, kind="ExternalInput")
    OUT = nc.dram_tensor("OUT", [128, NJ_pad // 2], BF16, kind="ExternalOutput")

    _ph = os.environ.get("KPARTS", "01B")
    nA0 = NS_pad // PAIR
    nSG = NS_pad // CH
    nBG = NJ_pad // CH if "B" in _ph else 0
    _do0, _do1 = "0" in _ph, "1" in _ph

    chunks = []
    q = 0
    while q < (NJ_pad if nBG else 0):
        n = min(GATH, NJ_pad - q)
        chunks.append((q, n))
        q += n
    nGC = len(chunks)

    with tile.TileContext(nc) as tc:
        with tc.tile_pool(name="const", bufs=1) as cst:
            _ska = tc.tile_pool(name="ska", bufs=1)
            sk = _ska.__enter__()
            _saa = tc.tile_pool(name="sa", bufs=2)
            sa = _saa.__enter__()
            _sk0 = tc.tile_pool(name="sk0", bufs=1)
            sk0 = _sk0.__enter__()

            # A0 inputs first so pair 0 can start early
            w0 = sk0.tile([64, K * 192], BF16)
            nc.sync.dma_start(w0[:], W0[:])
            x0sb = sk0.tile([64, NS_pad], BF16)
            nc.sync.dma_start(x0sb[:, 0:PAIR], X0T[:, 0:PAIR])
            nc.sync.dma_start(x0sb[:, PAIR:NS_pad // 2],
                              X0T[:, PAIR:NS_pad // 2])
            x1p = sk.tile([128, NS_pad // 2], BF16)
            nc.sync.dma_start(x1p[:, 0:NS_pad // 4], X1P[:, 0:NS_pad // 4])
            nc.sync.dma_start(x0sb[:, NS_pad // 2:], X0T[:, NS_pad // 2:])
            nc.sync.dma_start(x1p[:, NS_pad // 4:], X1P[:, NS_pad // 4:])
            whd = cst.tile([128, K * 256], BF16)
            nc.sync.dma_start(whd[:], WHD[:])
            w1i = cst.tile([128, K * 256], BF16)
            nc.sync.dma_start(w1i[:], W1I[:])
            wc2 = cst.tile([128, K * 64], BF16)
            nc.sync.dma_start(wc2[:], WC2[:])
            gidx = cst.tile([128, NJ_pad // 16], I16)
            nc.sync.dma_start(gidx[:], GIDX[:])

            # gather-native table: slot c -> partition c%128, elems
            # [(c//128)*128, +128) = [hx(64); cx(64)]
            tbl = cst.tile([128, NS_pad], BF16)

            # A0 outputs: hx1 halves and cx (cols indexed by pair)
            hx1sb = sk.tile([128, NS_pad // 2], BF16)
            cx0a = sk.tile([128, NS_pad // 2], BF16)

            # ---------------- A0: t0 (feature-major) --------------------
            s0s = [None] * nA0

            def _a0_cx(p):
                nc.vector.tensor_mul(cx0a[:, p * CH:(p + 1) * CH],
                                     s0s[p][:, 0:CH], s0s[p][:, CH:PAIR])

            def _a0_hx(p0, pn):
                t0t = sa.tile([128, pn * CH], BF16, tag="t0t",
                              padded_shape=[128, 2 * CH])
                nc.scalar.activation(t0t[:], cx0a[:, p0 * CH:(p0 + pn) * CH],
                                     TANH)
                for p in range(p0, p0 + pn):
                    tv = t0t[:, (p - p0) * CH:(p - p0 + 1) * CH]
                    nc.vector.tensor_mul(hx1sb[0:64, p * CH:(p + 1) * CH],
                                         s0s[p][0:64, PAIR:PAIR + CH],
                                         tv[0:64, :])
                    nc.vector.tensor_mul(hx1sb[64:128, p * CH:(p + 1) * CH],
                                         s0s[p][64:128, PAIR:PAIR + CH],
                                         tv[64:128, :])

            with tc.tile_pool(name="p0", bufs=2, space="PSUM") as p0:
                for p in range(nA0 if _do0 else 0):
                    w_lo = p * PAIR
                    t0g = p0.tile([128, PAIR + CH], F32, tag="t0")
                    for side in range(2):
                        pb = 64 * side
                        segs = _segments(t0b, w_lo + side * CH,
                                         w_lo + (side + 1) * CH)
                        for st, en, k0 in segs:
                            for g in range(3):
                                nc.tensor.matmul(
                                    t0g[pb:pb + 64, g * CH + st:g * CH + en],
                                    w0[:, 192 * k0 + 64 * g:192 * k0 + 64 * (g + 1)],
                                    x0sb[:, w_lo + side * CH + st:
                                         w_lo + side * CH + en],
                                    start=True, stop=True, tile_position=(0, pb))
                    s0 = sa.tile([128, PAIR + CH], BF16, tag="s0")
                    nc.scalar.activation(s0[:, 0:PAIR], t0g[:, 0:PAIR], SIG)
                    nc.scalar.activation(s0[:, PAIR:PAIR + CH],
                                         t0g[:, PAIR:PAIR + CH], SIG)
                    s0s[p] = s0
                    if p > 0:
                        _a0_cx(p - 1)
                    if p > 2 and p % 2 == 1:
                        _a0_hx(p - 3, 2)
                if _do0:
                    _a0_cx(nA0 - 1)
                    _a0_hx(nA0 - 2, 2)
            _sk0.__exit__(None, None, None)

            # B-phase SBUF pool opens now (reuses x0sb/w0 space); prefetch
            # the first x2 chunks early so phase B can start during A1.
            _sbb = tc.tile_pool(name="sb", bufs=2)
            sbp = _sbb.__enter__()

            def _x2dma(ci):
                q, n = chunks[ci]
                x2t = sbp.tile([64, n], BF16, tag="x2t", bufs=3,
                               name=f"x2t{ci}", padded_shape=[64, GATH])
                nc.sync.dma_start(x2t[:], X2T[:, q:q + n])
                return x2t

            x2ts = {ci: _x2dma(ci) for ci in range(min(2, nGC))}
            hxgs = {}

            def _gather(ci):
                q, n = chunks[ci]
                hxg = sbp.tile([128, 1, n], BF16, tag="hxg", bufs=3,
                               name=f"hxg{ci}", padded_shape=[128, 1, GATH])
                nc.gpsimd.dma_gather(
                    hxg[:], tbl[:, 0:lims[ci]],
                    gidx[:, q // 16:(q + n) // 16],
                    n, n, 128, transpose=True,
                    sbuf_tokens_per_rank=128,
                    sbuf_free_dim_per_rank=256)
                hxgs[ci] = hxg

            # shared A1/B PSUM pools: no psum barrier between phases
            _pg = tc.tile_pool(name="pg", bufs=3, space="PSUM")
            pg = _pg.__enter__()
            _pc = tc.tile_pool(name="pc", bufs=2, space="PSUM")
            pc = _pc.__enter__()

            # ---------------- A1: t1 (node-major, 2-acc gates) ----------
            sg1s = [None] * nSG
            pc1 = {}

            def _a1_mm(g):
                ps = pg.tile([128, GB * 256], F32, tag="g")
                pu = pc.tile([128, 512], F32, tag="c")
                p = g // 2
                pb = 64 * (g % 2)
                for i in range(GB):
                    b = g * GB + i
                    k1 = k1b[b]
                    nc.tensor.matmul(
                        ps[:, i * 256:(i + 1) * 256],
                        hx1sb[pb:pb + 64, p * CH + i * BLK:p * CH + (i + 1) * BLK],
                        whd[pb:pb + 64, 256 * k1:256 * (k1 + 1)],
                        start=True, stop=False)
                    nc.tensor.matmul(
                        ps[:, i * 256:(i + 1) * 256],
                        x1p[pb:pb + 64, p * CH + i * BLK:
                            p * CH + (i + 1) * BLK],
                        w1i[pb:pb + 64, 256 * k1:256 * (k1 + 1)],
                        start=False, stop=True)
                    nc.tensor.matmul(
                        pu[:, i * 64:(i + 1) * 64],
                        cx0a[pb:pb + 64, p * CH + i * BLK:
                             p * CH + (i + 1) * BLK],
                        wc2[pb:pb + 64, 64 * k1:64 * (k1 + 1)],
                        start=True, stop=True)
                pc1[g] = pu
                return ps

            def _a1_tail(g):
                sgg = sg1s[g][:].rearrange("p (b g) -> p b g", b=GB)
                pu = pc1.pop(g)
                ic = sa.tile([128, GB * 64], BF16, tag="ic1")
                icv = _r3(ic[:], GB)
                nc.vector.tensor_mul(icv, sgg[:, :, 0:64], sgg[:, :, 128:192])
                fcx = sa.tile([128, GB * 64], BF16, tag="fcx1")
                nc.vector.tensor_mul(_r3(fcx[:], GB), sgg[:, :, 64:128],
                                     _r3(pu[:, 0:GB * 64], GB))
                tv = tbl[:, g * CH:(g + 1) * CH].rearrange(
                    "p (b f) -> p b f", b=GB)
                nc.vector.tensor_add(tv[:, :, 64:128], icv, _r3(fcx[:], GB))

            def _a1_hx(g0, gn):
                tin = tbl[:, g0 * CH:(g0 + gn) * CH].rearrange(
                    "p (b f) -> p b f", b=GB * gn)
                t1t = sa.tile([128, GB * gn * 64], BF16, tag="t1t",
                              padded_shape=[128, GB * 4 * 64])
                nc.scalar.activation(_r3(t1t[:], GB * gn), tin[:, :, 64:128],
                                     TANH)
                for g in range(g0, g0 + gn):
                    sgg = sg1s[g][:].rearrange("p (b g) -> p b g", b=GB)
                    tv = tbl[:, g * CH:(g + 1) * CH].rearrange(
                        "p (b f) -> p b f", b=GB)
                    t1v = t1t[:, (g - g0) * 256:(g - g0 + 1) * 256]
                    nc.vector.tensor_mul(tv[:, :, 0:64], sgg[:, :, 192:256],
                                         _r3(t1v, GB))

            a1_done = 0     # groups whose hx/cx are fully in tbl
            gc_next = 0     # next gather chunk to issue

            def _issue_ready_gathers(cap):
                nonlocal gc_next
                while (nBG and gc_next < min(cap, nGC)
                       and lims[gc_next] <= a1_done * CH):
                    _gather(gc_next)
                    gc_next += 1

            pgs = {}
            for g in range(nSG if _do1 else 0):
                pgs[g] = _a1_mm(g)
                sg = sa.tile([128, GB * 256], BF16, tag="sg1", bufs=9)
                nc.scalar.activation(sg[:], pgs[g][:, 0:GB * 256], SIG)
                sg1s[g] = sg
                if g > 0:
                    _a1_tail(g - 1)
                    pgs.pop(g - 1)
                if g >= 4 + DLY and (g - DLY) % 4 == 0:
                    _a1_hx(g - DLY - 4, 4)
                    a1_done = g - DLY
                    _issue_ready_gathers(3)
            if _do1:
                _a1_tail(nSG - 1)
                pgs.pop(nSG - 1)
                while a1_done < nSG:
                    _a1_hx(a1_done, min(4, nSG - a1_done))
                    a1_done += min(4, nSG - a1_done)
            _issue_ready_gathers(3)

            # ---------------- B: t2 (node-major, 2-acc gates) -----------
            sg2s = {}
            cx2d = {}
            stage = {}
            pc2 = {}

            def _b_mm(g):
                ci = (g * CH) // GATH
                q, _ = chunks[ci]
                hxg, x2t = hxgs[ci], x2ts[ci]
                ps = pg.tile([128, GB * 256], F32, tag="g")
                pu = pc.tile([128, 512], F32, tag="c")
                for i in range(GB):
                    b = g * GB + i
                    k2 = k2b[b]
                    off = b * BLK - q
                    nc.tensor.matmul(
                        ps[:, i * 256:(i + 1) * 256],
                        hxg[0:64, 0, off:off + BLK],
                        whd[0:64, 256 * k2:256 * (k2 + 1)],
                        start=True, stop=False)
                    nc.tensor.matmul(
                        ps[:, i * 256:(i + 1) * 256],
                        x2t[:, off:off + BLK],
                        w1i[0:64, 256 * k2:256 * (k2 + 1)],
                        start=False, stop=True)
                    nc.tensor.matmul(
                        pu[:, i * 64:(i + 1) * 64],
                        hxg[64:128, 0, off:off + BLK],
                        wc2[64:128, 64 * k2:64 * (k2 + 1)],
                        start=True, stop=True)
                pc2[g] = pu
                return ps

            def _b_tail(g):
                sgg = sg2s[g][:].rearrange("p (b g) -> p b g", b=GB)
                pu = pc2.pop(g)
                ic = sbp.tile([128, GB * 64], BF16, tag="ic2")
                icv = _r3(ic[:], GB)
                nc.vector.tensor_mul(icv, sgg[:, :, 0:64], sgg[:, :, 128:192])
                fcx = sbp.tile([128, GB * 64], BF16, tag="fcx2")
                nc.vector.tensor_mul(_r3(fcx[:], GB), sgg[:, :, 64:128],
                                     _r3(pu[:, 0:GB * 64], GB))
                cx2s = cx2d[g // TBAT]
                cv = cx2s[:, (g % TBAT) * 256:(g % TBAT + 1) * 256]
                nc.vector.tensor_add(_r3(cv, GB), icv, _r3(fcx[:], GB))

            def _b_hx(g0, gn):
                bb = g0 // TBAT
                cx2s = cx2d[bb]
                off = g0 % TBAT
                if off + gn == TBAT or g0 + gn == nBG:
                    cx2d.pop(bb)
                t2t = sbp.tile([128, gn * 256], BF16, tag="t2t",
                               padded_shape=[128, TBAT * 256])
                nc.scalar.activation(
                    t2t[:], cx2s[:, off * 256:(off + gn) * 256], TANH)
                for g in range(g0, g0 + gn):
                    sgg = sg2s[g][:].rearrange("p (b g) -> p b g", b=GB)
                    sv = stage[g // STAGE_G][
                        :, (g % STAGE_G) * 256:(g % STAGE_G + 1) * 256]
                    t2v = t2t[:, (g - g0) * 256:(g - g0 + 1) * 256]
                    nc.vector.tensor_mul(_r3(sv, GB), sgg[:, :, 192:256],
                                         _r3(t2v, GB))
                    sg2s.pop(g)
                # flush only fully-completed stage chunks
                for c in range(g0 // STAGE_G, (g0 + gn) // STAGE_G):
                    st = stage.pop(c)
                    nc.gpsimd.dma_start(
                        OUT[:, c * 1024:(c + 1) * 1024], st[:])

            b_done = 0
            for g in range(nBG):
                ci = (g * CH) // GATH
                if g * CH == chunks[ci][0]:
                    while gc_next < min(ci + 3, nGC):
                        _gather(gc_next)
                        gc_next += 1
                    for cx in range(ci + 1, min(ci + 3, nGC)):
                        if cx not in x2ts:
                            x2ts[cx] = _x2dma(cx)
                if g % TBAT == 0:
                    cx2d[g // TBAT] = sbp.tile(
                        [128, TBAT * 256], BF16, tag="cx2s",
                        name=f"cx2_{(g // TBAT) % 2}")
                if g % STAGE_G == 0:
                    stage[g // STAGE_G] = sbp.tile(
                        [128, STAGE_G * 256], BF16, tag="stage", bufs=4,
                        name=f"stg{g // STAGE_G}")
                psg = _b_mm(g)
                sg = sbp.tile([128, GB * 256], BF16, tag="sg2",
                              bufs=TBAT + DLY + 2,
                              name=f"sg2_{g % (TBAT + DLY + 2)}")
                nc.scalar.activation(sg[:], psg[:, 0:GB * 256], SIG)
                sg2s[g] = sg
                if g > 0:
                    _b_tail(g - 1)
                if g >= TBAT + DLY and (g - DLY) % TBAT == 0:
                    _b_hx(g - DLY - TBAT, TBAT)
                    b_done = g - DLY
                # free fully-consumed chunk tiles
                for cc in [c for c in hxgs if c < ci]:
                    if (chunks[cc][0] + chunks[cc][1]) <= g * CH:
                        hxgs.pop(cc)
                        x2ts.pop(cc, None)
            if nBG:
                _b_tail(nBG - 1)
                while b_done < nBG:
                    gn = min(2, nBG - b_done, TBAT - b_done % TBAT)
                    _b_hx(b_done, gn)
                    b_done += gn

            _pc.__exit__(None, None, None)
            _pg.__exit__(None, None, None)
            _sbb.__exit__(None, None, None)
            _saa.__exit__(None, None, None)
            _ska.__exit__(None, None, None)

    nc.finalize()
    return nc


LAST_NC = None


def _run(inputs, trace=False):
    global LAST_NC
    in_maps, meta = _prepare(**inputs)
    nc = _build(meta)
    LAST_NC = nc
    res = run_bass_kernel_spmd(nc, in_maps, core_ids=list(range(NCORES)),
                               trace=trace)
    NJ_pad = meta["NJ_pad"]
    out = np.zeros((N3, 64), np.float32)
    nCh = NJ_pad // (STAGE_G * GB * BLK)
    for c in range(NCORES):
        jslots, jids = meta["j_slot_lists"][c]
        oc = res.results[c]["OUT"]          # [128, NJ_pad//2] bf16
        # stage chunk ch, block b (16 per chunk), partition p ->
        #   node slot ch*2048 + b*128 + p, feats at col ch*1024 + b*64
        arr = oc.reshape(128, nCh, 16, 64).transpose(1, 2, 0, 3)
        arr = np.ascontiguousarray(arr).reshape(NJ_pad, 64)
        out[jids] = arr[jslots].astype(np.float32)
    return out, res


def kernel(**inputs):
    out, _ = _run(inputs, trace=False)
    return out
